# revision 1
# baseline (speedup 1.0000x reference)
"""Trainium2 Bass kernel for nn_MultiHeadAttention_64037962383811.

Reference (per batch b):
  q = x @ Wq[h].T + bq[h];  k = states @ Wk[h].T + bk[h];  v = states @ Wv[h].T + bv[h]
  scores = q k^T / sqrt(512);  masked softmax over Lk;  ctx = attn @ v
  out = concat_h(ctx) @ Wp.T + bp

Sharding: data-parallel over batch B=8 -> one batch element per NeuronCore
(8 cores). No collectives; each core computes its full [1024, 512] output
slice and the host stacks them.

Per-core dataflow (all matmuls in float32r = TF32-class, full PE rate):
  - Everything is kept in "transposed" layouts so that the PE's
    partition-dim contraction lines up with zero on-chip transposes:
      x^T, states^T        [e, l]   (host-transposed)
      Q^T, K^T = W^T @ x^T [d, l]   (from projection matmuls directly)
      S^T = K Q^T          [k, q]   (scores, transposed)
      P^T = exp(S^T) * m^T          (mask host-transposed, bf16)
      rowsum = ones^T @ P^T  [1, q] (partition-dim reduce on PE)
      ctx^T = V^T P^T      [d, q]   (V kept [k, d], natural)
      out = ctx_n^T.T @ Wp^T [q, o] (accumulated over heads in SBUF)
  - Softmax without max-subtraction (scores ~ N(0,1), exp is safe) and
    without -inf masking: P = exp(S) * mask, normalized by rowsum(P).
  - Division via reciprocal_approx_fast + gpsimd partition-broadcast.

The head loop is software-pipelined: iteration h emits [proj(h),
outproj(h-1), attn(h)] so the PE never waits on the softmax/normalize
tail of the previous head.
"""
import sys

for _p in (
    "/root/.axon_site",
    "/root/.axon_site/_ro/trn_rl_repo",
    "/root/.axon_site/_ro/pypackages",
):
    if _p not in sys.path:
        sys.path.insert(0, _p)

import numpy as np
import ml_dtypes
from contextlib import ExitStack

import concourse.bacc as bacc
import concourse.tile as tile
import concourse.mybir as mybir
from concourse.bass_utils import run_bass_kernel_spmd

B, L, E, D, H = 8, 1024, 512, 512, 8
NCORES = 8
F32 = mybir.dt.float32
F32R = mybir.dt.float32r
BF16 = mybir.dt.bfloat16
AF = mybir.ActivationFunctionType
SCALE = float(1.0 / np.sqrt(E))

PT_BUFS = 8  # P^T sbuf tiles in flight (8 needed live per (h, qb))

TRACE = False  # test harness sets kernel.TRACE = True to profile
LAST_EXEC_NS = None

_cache = {}


def _build():
    nc = bacc.Bacc("TRN2", target_bir_lowering=False, debug=False)

    xT_d = nc.dram_tensor("xT", [4, 128, L], F32R, kind="ExternalInput").ap()
    sT_d = nc.dram_tensor("sT", [4, 128, L], F32R, kind="ExternalInput").ap()
    mk_d = nc.dram_tensor("maskT", [8, 128, L], BF16, kind="ExternalInput").ap()
    wq_d = nc.dram_tensor("wqT", [H, 4, 128, D], F32R, kind="ExternalInput").ap()
    wk_d = nc.dram_tensor("wkT", [H, 4, 128, D], F32R, kind="ExternalInput").ap()
    wv_d = nc.dram_tensor("wvT", [H, 4, 128, D], F32R, kind="ExternalInput").ap()
    wp_d = nc.dram_tensor("wpT", [H, 4, 128, D], F32R, kind="ExternalInput").ap()
    bq_d = nc.dram_tensor("bqT", [H, 128, 4], F32, kind="ExternalInput").ap()
    bk_d = nc.dram_tensor("bkT", [H, 128, 4], F32, kind="ExternalInput").ap()
    bv_d = nc.dram_tensor("bv", [H, D], F32R, kind="ExternalInput").ap()
    bp_d = nc.dram_tensor("bp", [1, D], F32R, kind="ExternalInput").ap()
    on_d = nc.dram_tensor("ones", [128, 128], F32R, kind="ExternalInput").ap()
    out_d = nc.dram_tensor("out", [L, D], F32, kind="ExternalOutput").ap()

    with tile.TileContext(nc) as tc, ExitStack() as ctx:
        const = ctx.enter_context(tc.tile_pool(name="const", bufs=1))
        wpool = ctx.enter_context(tc.tile_pool(name="w", bufs=1))
        qkv = ctx.enter_context(tc.tile_pool(name="qkv", bufs=1))
        ptp = ctx.enter_context(tc.tile_pool(name="ptp", bufs=PT_BUFS))
        ctxp = ctx.enter_context(tc.tile_pool(name="ctxp", bufs=1))
        small = ctx.enter_context(tc.tile_pool(name="small", bufs=2))
        psum = ctx.enter_context(tc.tile_pool(name="ps", bufs=7, space="PSUM"))
        psrow = ctx.enter_context(tc.tile_pool(name="psrow", bufs=1, space="PSUM"))

        # ---- resident loads -------------------------------------------------
        ones = const.tile([128, 128], F32R, tag="ones")
        nc.sync.dma_start(ones[:], on_d)
        mask_sb = const.tile([128, 8, L], BF16, tag="mask")
        for kj in range(8):
            nc.sync.dma_start(mask_sb[:, kj, :], mk_d[kj])
        xT = const.tile([128, 4, L], F32R, tag="xT")
        sT = const.tile([128, 4, L], F32R, tag="sT")
        for ej in range(4):
            nc.sync.dma_start(xT[:, ej, :], xT_d[ej])
            nc.sync.dma_start(sT[:, ej, :], sT_d[ej])
        bq_sb = const.tile([128, H, 4], F32, tag="bq")
        bk_sb = const.tile([128, H, 4], F32, tag="bk")
        for h in range(H):
            nc.sync.dma_start(bq_sb[:, h, :], bq_d[h])
            nc.sync.dma_start(bk_sb[:, h, :], bk_d[h])
        bp_sb = const.tile([1, D], F32R, tag="bp")
        nc.sync.dma_start(bp_sb[:], bp_d)
        out_acc = const.tile([128, 8, D], F32, tag="oacc")

        state = {}

        def proj(h):
            """Q^T, K^T [128,4dj,L] and V [128,8kj,D] projections for head h."""
            wq = wpool.tile([128, 4, D], F32R, tag="wq")
            wk = wpool.tile([128, 4, D], F32R, tag="wk")
            wv = wpool.tile([128, 4, D], F32R, tag="wv")
            for ej in range(4):
                nc.sync.dma_start(wq[:, ej, :], wq_d[h, ej])
                nc.sync.dma_start(wk[:, ej, :], wk_d[h, ej])
                nc.sync.dma_start(wv[:, ej, :], wv_d[h, ej])
            bv_t = small.tile([1, D], F32R, tag="bv")
            nc.sync.dma_start(bv_t[:], bv_d[h : h + 1, :])

            qt = qkv.tile([128, 4, L], F32R, tag="qt")
            kt = qkv.tile([128, 4, L], F32R, tag="kt")
            vt = qkv.tile([128, 8, D], F32R, tag="vt")
            # Q^T / K^T: out[d_tile, q] = sum_e W^T[e, d].T @ xT[e, q]
            for wmat, src, dst, bias in (
                (wq, xT, qt, bq_sb),
                (wk, sT, kt, bk_sb),
            ):
                for qb in range(2):
                    for dj in range(4):
                        ps = psum.tile([128, 512], F32, tag="mm")
                        for ej in range(4):
                            nc.tensor.matmul(
                                ps[:],
                                wmat[:, ej, dj * 128 : (dj + 1) * 128],
                                src[:, ej, qb * 512 : (qb + 1) * 512],
                                start=(ej == 0),
                                stop=(ej == 3),
                            )
                        nc.scalar.activation(
                            dst[:, dj, qb * 512 : (qb + 1) * 512],
                            ps[:],
                            AF.Identity,
                            bias=bias[:, h, dj : dj + 1],
                        )
            # V: out[k_tile, d] = sum_e sT[e, k].T @ Wv^T[e, d]  (+ bv via ones)
            for kj in range(8):
                ps = psum.tile([128, 512], F32, tag="mm")
                for ej in range(4):
                    nc.tensor.matmul(
                        ps[:],
                        sT[:, ej, kj * 128 : (kj + 1) * 128],
                        wv[:, ej, :],
                        start=(ej == 0),
                        stop=False,
                    )
                nc.tensor.matmul(
                    ps[:], ones[0:1, :], bv_t[:], start=False, stop=True
                )
                nc.scalar.copy(vt[:, kj, :], ps[:])
            state[h] = {"qt": qt, "kt": kt, "vt": vt}

        def attn(h):
            """S^T -> exp*mask -> rowsum -> ctx^T -> normalize, per q-block."""
            st = state[h]
            qt, kt, vt = st["qt"], st["kt"], st["vt"]
            ctxn = ctxp.tile([128, 4, L], F32R, tag="ctxn")
            for qb in range(2):
                qsl = slice(qb * 512, (qb + 1) * 512)
                pts = []
                for kj in range(8):
                    ps = psum.tile([128, 512], F32, tag="mm")
                    for dc in range(4):
                        nc.tensor.matmul(
                            ps[:],
                            kt[:, dc, kj * 128 : (kj + 1) * 128],
                            qt[:, dc, qsl],
                            start=(dc == 0),
                            stop=(dc == 3),
                        )
                    pt = ptp.tile([128, 512], F32R, tag="pt")
                    nc.scalar.activation(pt[:], ps[:], AF.Exp, scale=SCALE)
                    nc.vector.tensor_mul(pt[:], pt[:], mask_sb[:, kj, qsl])
                    pts.append(pt)
                rs = psrow.tile([1, 512], F32, tag="row")
                for kj in range(8):
                    nc.tensor.matmul(
                        rs[:],
                        ones[:, 0:1],
                        pts[kj][:],
                        start=(kj == 0),
                        stop=(kj == 7),
                    )
                rec = small.tile([1, 512], F32, tag="rec")
                nc.vector.reciprocal_approx_fast(rec[:], rs[:])
                rb = small.tile([128, 512], F32, tag="rb")
                nc.gpsimd.partition_broadcast(rb[:], rec[:])
                for dj in range(4):
                    cps = psum.tile([128, 512], F32, tag="mm")
                    for kj in range(8):
                        nc.tensor.matmul(
                            cps[:],
                            vt[:, kj, dj * 128 : (dj + 1) * 128],
                            pts[kj][:],
                            start=(kj == 0),
                            stop=(kj == 7),
                        )
                    nc.vector.tensor_mul(ctxn[:, dj, qsl], cps[:], rb[:])
            state[h]["ctxn"] = ctxn

        def outproj(h):
            """out_acc[q, o] += sum_dj ctx_n^T[i, q].T @ Wp^T[i, o]."""
            wp = wpool.tile([128, 4, D], F32R, tag="wp")
            for dj in range(4):
                nc.sync.dma_start(wp[:, dj, :], wp_d[h, dj])
            ctxn = state[h]["ctxn"]
            for qm in range(8):
                ps = psum.tile([128, 512], F32, tag="mm")
                for dj in range(4):
                    nc.tensor.matmul(
                        ps[:],
                        ctxn[:, dj, qm * 128 : (qm + 1) * 128],
                        wp[:, dj, :],
                        start=(dj == 0),
                        stop=(dj == 3 and h != 0),
                    )
                if h == 0:
                    nc.tensor.matmul(
                        ps[:], ones[0:1, :], bp_sb[:], start=False, stop=True
                    )
                    nc.scalar.copy(out_acc[:, qm, :], ps[:])
                else:
                    nc.vector.tensor_add(
                        out_acc[:, qm, :], out_acc[:, qm, :], ps[:]
                    )
            # free per-head references
            del state[h]["qt"], state[h]["kt"], state[h]["vt"], state[h]["ctxn"]

        for h in range(H):
            proj(h)
            if h > 0:
                outproj(h - 1)
            attn(h)
        outproj(H - 1)

        nc.sync.dma_start(
            out_d.rearrange("(qm p) o -> p qm o", p=128), out_acc[:]
        )

    nc.compile()
    return nc


def _get_program():
    if "nc" not in _cache:
        _cache["nc"] = _build()
    return _cache["nc"]


def kernel(x, states, mask, Wq, bq, Wk, bk, Wv, bv, Wp, bp):
    global LAST_EXEC_NS
    nc = _get_program()

    x = np.asarray(x, dtype=np.float32)
    states = np.asarray(states, dtype=np.float32)
    mask = np.asarray(mask)
    f32 = np.float32
    wq_np = np.ascontiguousarray(
        np.asarray(Wq, f32).transpose(0, 2, 1)
    ).reshape(H, 4, 128, D)
    wk_np = np.ascontiguousarray(
        np.asarray(Wk, f32).transpose(0, 2, 1)
    ).reshape(H, 4, 128, D)
    wv_np = np.ascontiguousarray(
        np.asarray(Wv, f32).transpose(0, 2, 1)
    ).reshape(H, 4, 128, D)
    wp_np = np.ascontiguousarray(np.asarray(Wp, f32).T).reshape(H, 4, 128, D)
    bq_np = np.ascontiguousarray(
        np.asarray(bq, f32).reshape(H, 4, 128).transpose(0, 2, 1)
    )
    bk_np = np.ascontiguousarray(
        np.asarray(bk, f32).reshape(H, 4, 128).transpose(0, 2, 1)
    )
    bv_np = np.asarray(bv, f32)
    bp_np = np.asarray(bp, f32).reshape(1, D)
    ones_np = np.ones((128, 128), f32)

    shared = {
        "wqT": wq_np,
        "wkT": wk_np,
        "wvT": wv_np,
        "wpT": wp_np,
        "bqT": bq_np,
        "bkT": bk_np,
        "bv": bv_np,
        "bp": bp_np,
        "ones": ones_np,
    }
    in_maps = []
    for b in range(B):
        xT = np.ascontiguousarray(x[b].T).reshape(4, 128, L)
        sT = np.ascontiguousarray(states[b].T).reshape(4, 128, L)
        mT = np.ascontiguousarray(mask[b].T).astype(ml_dtypes.bfloat16).reshape(
            8, 128, L
        )
        in_maps.append({"xT": xT, "sT": sT, "maskT": mT, **shared})

    res = run_bass_kernel_spmd(
        nc, in_maps, core_ids=list(range(NCORES)), trace=TRACE
    )
    LAST_EXEC_NS = res.exec_time_ns
    return np.stack([res.results[b]["out"] for b in range(B)], axis=0)


# revision 2
# speedup vs baseline: 1.1512x; 1.1512x over previous
"""Trainium2 Bass kernel for nn_MultiHeadAttention_64037962383811.

Reference (per batch b):
  q = x @ Wq[h].T + bq[h];  k = states @ Wk[h].T + bk[h];  v = states @ Wv[h].T + bv[h]
  scores = q k^T / sqrt(512);  masked softmax over Lk;  ctx = attn @ v
  out = concat_h(ctx) @ Wp.T + bp

Sharding: data-parallel over batch B=8 -> one batch element per NeuronCore
(8 cores). No collectives; each core computes its full [1024, 512] output
slice and the host stacks them.

Per-core dataflow (all matmuls in float32r = TF32-class, full PE rate):
  - Everything is kept in "transposed" layouts so that the PE's
    partition-dim contraction lines up with zero on-chip transposes:
      x^T, states^T        [e, l]   (host-transposed)
      Q^T, K^T = W^T @ x^T [d, l]   (from projection matmuls directly)
      S^T = K Q^T          [k, q]   (scores, transposed)
      P^T = exp(S^T) * m^T          (mask host-transposed, bf16)
      rowsum = ones^T @ P^T  [1, q] (partition-dim reduce on PE)
      ctx^T = V^T P^T      [d, q]   (V kept [k, d], natural)
      out = ctx_n^T.T @ Wp^T [q, o] (accumulated over heads in SBUF)
  - Softmax without max-subtraction (scores ~ N(0,1), exp is safe) and
    without -inf masking: P = exp(S) * mask, normalized by rowsum(P).
  - Division via reciprocal_approx_fast + gpsimd partition-broadcast.

The head loop is software-pipelined: iteration h emits [proj(h),
outproj(h-1), attn(h)] so the PE never waits on the softmax/normalize
tail of the previous head. Bias matmuls are compiled out when all bias
vectors are zero (they are, for this problem's setup_inputs).
"""
import sys

for _p in (
    "/root/.axon_site",
    "/root/.axon_site/_ro/trn_rl_repo",
    "/root/.axon_site/_ro/pypackages",
):
    if _p not in sys.path:
        sys.path.insert(0, _p)

import numpy as np
import ml_dtypes
from contextlib import ExitStack

import concourse.bacc as bacc
import concourse.tile as tile
import concourse.mybir as mybir
from concourse.bass_utils import run_bass_kernel_spmd

B, L, E, D, H = 8, 1024, 512, 512, 8
NCORES = 8
F32 = mybir.dt.float32
F32R = mybir.dt.float32r
BF16 = mybir.dt.bfloat16
AF = mybir.ActivationFunctionType
SCALE = float(1.0 / np.sqrt(E))

PT_BUFS = 8  # P^T sbuf tiles in flight (8 needed live per (h, qb))

TRACE = False  # test harness sets kernel.TRACE = True to profile
LAST_EXEC_NS = None

_cache = {}


def _build(use_bias):
    nc = bacc.Bacc("TRN2", target_bir_lowering=False, debug=False)

    xT_d = nc.dram_tensor("xT", [4, 128, L], F32R, kind="ExternalInput").ap()
    sT_d = nc.dram_tensor("sT", [4, 128, L], F32R, kind="ExternalInput").ap()
    mk_d = nc.dram_tensor("maskT", [8, 128, L], BF16, kind="ExternalInput").ap()
    wq_d = nc.dram_tensor("wqT", [H, 4, 128, D], F32R, kind="ExternalInput").ap()
    wk_d = nc.dram_tensor("wkT", [H, 4, 128, D], F32R, kind="ExternalInput").ap()
    wv_d = nc.dram_tensor("wvT", [H, 4, 128, D], F32R, kind="ExternalInput").ap()
    wp_d = nc.dram_tensor("wpT", [H, 4, 128, D], F32R, kind="ExternalInput").ap()
    if use_bias:
        bq_d = nc.dram_tensor("bqT", [H, 128, 4], F32, kind="ExternalInput").ap()
        bk_d = nc.dram_tensor("bkT", [H, 128, 4], F32, kind="ExternalInput").ap()
        bv_d = nc.dram_tensor("bv", [H, D], F32R, kind="ExternalInput").ap()
        bp_d = nc.dram_tensor("bp", [1, D], F32R, kind="ExternalInput").ap()
    on_d = nc.dram_tensor("ones", [128, 128], F32R, kind="ExternalInput").ap()
    out_d = nc.dram_tensor("out", [L, D], F32, kind="ExternalOutput").ap()

    with tile.TileContext(nc) as tc, ExitStack() as ctx:
        const = ctx.enter_context(tc.tile_pool(name="const", bufs=1))
        wpool = ctx.enter_context(tc.tile_pool(name="w", bufs=1))
        qkv = ctx.enter_context(tc.tile_pool(name="qkv", bufs=1))
        ptp = ctx.enter_context(tc.tile_pool(name="ptp", bufs=PT_BUFS))
        ctxp = ctx.enter_context(tc.tile_pool(name="ctxp", bufs=1))
        small = ctx.enter_context(tc.tile_pool(name="small", bufs=2))
        psum = ctx.enter_context(tc.tile_pool(name="ps", bufs=7, space="PSUM"))
        psrow = ctx.enter_context(tc.tile_pool(name="psrow", bufs=1, space="PSUM"))

        # Resident tiles. DMA emission order matters for startup latency:
        # head-0 weights + xT/sT go first, bulky mask/bias loads after the
        # first projections are emitted.
        mask_sb = const.tile([128, 8, L], BF16, tag="mask")
        xT = const.tile([128, 4, L], F32R, tag="xT")
        sT = const.tile([128, 4, L], F32R, tag="sT")
        ones = const.tile([128, 128], F32R, tag="ones")
        out_acc = const.tile([128, 8, D], F32, tag="oacc")
        if use_bias:
            bq_sb = const.tile([128, H, 4], F32, tag="bq")
            bk_sb = const.tile([128, H, 4], F32, tag="bk")
            bp_sb = const.tile([1, D], F32R, tag="bp")

        def load_consts():
            """Emitted after proj(0): non-critical-path resident loads."""
            for kj in range(8):
                nc.sync.dma_start(mask_sb[:, kj, :], mk_d[kj])
            nc.sync.dma_start(ones[:], on_d)
            if use_bias:
                for h in range(H):
                    nc.sync.dma_start(bq_sb[:, h, :], bq_d[h])
                    nc.sync.dma_start(bk_sb[:, h, :], bk_d[h])
                nc.sync.dma_start(bp_sb[:], bp_d)

        state = {}

        def proj(h):
            """Q^T, K^T [128,4dj,L] and V [128,8kj,D] projections for head h."""
            wq = wpool.tile([128, 4, D], F32R, tag="wq")
            wk = wpool.tile([128, 4, D], F32R, tag="wk")
            wv = wpool.tile([128, 4, D], F32R, tag="wv")
            for ej in range(4):
                nc.sync.dma_start(wq[:, ej, :], wq_d[h, ej])
                if h == 0:
                    nc.sync.dma_start(xT[:, ej, :], xT_d[ej])
            for ej in range(4):
                nc.sync.dma_start(wk[:, ej, :], wk_d[h, ej])
                if h == 0:
                    nc.sync.dma_start(sT[:, ej, :], sT_d[ej])
            for ej in range(4):
                nc.sync.dma_start(wv[:, ej, :], wv_d[h, ej])
            if use_bias:
                bv_t = small.tile([1, D], F32R, tag="bv")
                nc.sync.dma_start(bv_t[:], bv_d[h : h + 1, :])

            qt = qkv.tile([128, 4, L], F32R, tag="qt")
            kt = qkv.tile([128, 4, L], F32R, tag="kt")
            vt = qkv.tile([128, 8, D], F32R, tag="vt")
            # Q^T / K^T: out[d_tile, q] = sum_e W^T[e, d].T @ xT[e, q]
            for wmat, src, dst, which in (
                (wq, xT, qt, "q"),
                (wk, sT, kt, "k"),
            ):
                for qb in range(2):
                    for dj in range(4):
                        ps = psum.tile([128, 512], F32, tag="mm")
                        for ej in range(4):
                            nc.tensor.matmul(
                                ps[:],
                                wmat[:, ej, dj * 128 : (dj + 1) * 128],
                                src[:, ej, qb * 512 : (qb + 1) * 512],
                                start=(ej == 0),
                                stop=(ej == 3),
                            )
                        dsl = dst[:, dj, qb * 512 : (qb + 1) * 512]
                        if use_bias:
                            bias = bq_sb if which == "q" else bk_sb
                            nc.scalar.activation(
                                dsl, ps[:], AF.Identity,
                                bias=bias[:, h, dj : dj + 1],
                            )
                        else:
                            nc.scalar.copy(dsl, ps[:])
            # V: out[k_tile, d] = sum_e sT[e, k].T @ Wv^T[e, d]  (+ bv via ones)
            for kj in range(8):
                ps = psum.tile([128, 512], F32, tag="mm")
                for ej in range(4):
                    nc.tensor.matmul(
                        ps[:],
                        sT[:, ej, kj * 128 : (kj + 1) * 128],
                        wv[:, ej, :],
                        start=(ej == 0),
                        stop=(ej == 3 and not use_bias),
                    )
                if use_bias:
                    nc.tensor.matmul(
                        ps[:], ones[0:1, :], bv_t[:], start=False, stop=True
                    )
                nc.scalar.copy(vt[:, kj, :], ps[:])
            state[h] = {"qt": qt, "kt": kt, "vt": vt}

        def attn(h):
            """S^T -> exp*mask -> rowsum -> ctx^T -> normalize, per q-block."""
            st = state[h]
            qt, kt, vt = st["qt"], st["kt"], st["vt"]
            ctxn = ctxp.tile([128, 4, L], F32R, tag="ctxn")
            for qb in range(2):
                qsl = slice(qb * 512, (qb + 1) * 512)
                pts = []
                for kj in range(8):
                    ps = psum.tile([128, 512], F32, tag="mm")
                    for dc in range(4):
                        nc.tensor.matmul(
                            ps[:],
                            kt[:, dc, kj * 128 : (kj + 1) * 128],
                            qt[:, dc, qsl],
                            start=(dc == 0),
                            stop=(dc == 3),
                        )
                    pt = ptp.tile([128, 512], F32R, tag="pt")
                    nc.scalar.activation(pt[:], ps[:], AF.Exp, scale=SCALE)
                    nc.vector.tensor_mul(pt[:], pt[:], mask_sb[:, kj, qsl])
                    pts.append(pt)
                rs = psrow.tile([1, 512], F32, tag="row")
                for kj in range(8):
                    nc.tensor.matmul(
                        rs[:],
                        ones[:, 0:1],
                        pts[kj][:],
                        start=(kj == 0),
                        stop=(kj == 7),
                    )
                rec = small.tile([1, 512], F32, tag="rec")
                nc.vector.reciprocal_approx_fast(rec[:], rs[:])
                rb = small.tile([128, 512], F32, tag="rb")
                nc.gpsimd.partition_broadcast(rb[:], rec[:])
                for dj in range(4):
                    cps = psum.tile([128, 512], F32, tag="mm")
                    for kj in range(8):
                        nc.tensor.matmul(
                            cps[:],
                            vt[:, kj, dj * 128 : (dj + 1) * 128],
                            pts[kj][:],
                            start=(kj == 0),
                            stop=(kj == 7),
                        )
                    nc.vector.tensor_mul(ctxn[:, dj, qsl], cps[:], rb[:])
            state[h]["ctxn"] = ctxn

        def outproj(h):
            """out_acc[q, o] += sum_dj ctx_n^T[i, q].T @ Wp^T[i, o]."""
            wp = wpool.tile([128, 4, D], F32R, tag="wp")
            for dj in range(4):
                nc.sync.dma_start(wp[:, dj, :], wp_d[h, dj])
            ctxn = state[h]["ctxn"]
            for qm in range(8):
                ps = psum.tile([128, 512], F32, tag="mm")
                for dj in range(4):
                    nc.tensor.matmul(
                        ps[:],
                        ctxn[:, dj, qm * 128 : (qm + 1) * 128],
                        wp[:, dj, :],
                        start=(dj == 0),
                        stop=(dj == 3 and not (h == 0 and use_bias)),
                    )
                if h == 0 and use_bias:
                    nc.tensor.matmul(
                        ps[:], ones[0:1, :], bp_sb[:], start=False, stop=True
                    )
                if h == 0:
                    nc.scalar.copy(out_acc[:, qm, :], ps[:])
                else:
                    nc.vector.tensor_add(
                        out_acc[:, qm, :], out_acc[:, qm, :], ps[:]
                    )
                if h == H - 1:
                    nc.sync.dma_start(
                        out_d[qm * 128 : (qm + 1) * 128, :], out_acc[:, qm, :]
                    )
            del state[h]["qt"], state[h]["kt"], state[h]["vt"], state[h]["ctxn"]

        for h in range(H):
            proj(h)
            if h == 0:
                load_consts()
            if h > 0:
                outproj(h - 1)
            attn(h)
        outproj(H - 1)

    nc.compile()
    return nc


def _get_program(use_bias):
    key = ("nc", use_bias)
    if key not in _cache:
        _cache[key] = _build(use_bias)
    return _cache[key]


def kernel(x, states, mask, Wq, bq, Wk, bk, Wv, bv, Wp, bp):
    global LAST_EXEC_NS

    x = np.asarray(x, dtype=np.float32)
    states = np.asarray(states, dtype=np.float32)
    mask = np.asarray(mask)
    f32 = np.float32
    bq_np, bk_np = np.asarray(bq, f32), np.asarray(bk, f32)
    bv_np, bp_np = np.asarray(bv, f32), np.asarray(bp, f32)
    use_bias = bool(
        bq_np.any() or bk_np.any() or bv_np.any() or bp_np.any()
    )
    nc = _get_program(use_bias)

    wq_np = np.ascontiguousarray(
        np.asarray(Wq, f32).transpose(0, 2, 1)
    ).reshape(H, 4, 128, D)
    wk_np = np.ascontiguousarray(
        np.asarray(Wk, f32).transpose(0, 2, 1)
    ).reshape(H, 4, 128, D)
    wv_np = np.ascontiguousarray(
        np.asarray(Wv, f32).transpose(0, 2, 1)
    ).reshape(H, 4, 128, D)
    wp_np = np.ascontiguousarray(np.asarray(Wp, f32).T).reshape(H, 4, 128, D)

    shared = {
        "wqT": wq_np,
        "wkT": wk_np,
        "wvT": wv_np,
        "wpT": wp_np,
        "ones": np.ones((128, 128), f32),
    }
    if use_bias:
        shared["bqT"] = np.ascontiguousarray(
            bq_np.reshape(H, 4, 128).transpose(0, 2, 1)
        )
        shared["bkT"] = np.ascontiguousarray(
            bk_np.reshape(H, 4, 128).transpose(0, 2, 1)
        )
        shared["bv"] = bv_np
        shared["bp"] = bp_np.reshape(1, D)

    in_maps = []
    for b in range(B):
        xT = np.ascontiguousarray(x[b].T).reshape(4, 128, L)
        sT = np.ascontiguousarray(states[b].T).reshape(4, 128, L)
        mT = np.ascontiguousarray(mask[b].T).astype(ml_dtypes.bfloat16).reshape(
            8, 128, L
        )
        in_maps.append({"xT": xT, "sT": sT, "maskT": mT, **shared})

    res = run_bass_kernel_spmd(
        nc, in_maps, core_ids=list(range(NCORES)), trace=TRACE
    )
    LAST_EXEC_NS = res.exec_time_ns
    return np.stack([res.results[b]["out"] for b in range(B)], axis=0)


# revision 8
# speedup vs baseline: 1.2117x; 1.0526x over previous
"""Trainium2 Bass kernel for nn_MultiHeadAttention_64037962383811.

Reference (per batch b):
  q = x @ Wq[h].T + bq[h];  k = states @ Wk[h].T + bk[h];  v = states @ Wv[h].T + bv[h]
  scores = q k^T / sqrt(512);  masked softmax over Lk;  ctx = attn @ v
  out = concat_h(ctx) @ Wp.T + bp

Sharding: data-parallel over batch B=8 -> one batch element per NeuronCore
(8 cores). No collectives; each core computes its full [1024, 512] output
slice and the host stacks them.

Per-core dataflow (all matmuls in float32r = TF32-class, full PE rate):
  - Everything is kept in "transposed" layouts so that the PE's
    partition-dim contraction lines up with zero on-chip transposes:
      x^T, states^T        [e, l]   (host-transposed)
      Q^T, K^T = W^T @ x^T [d, l]   (from projection matmuls directly)
      S^T = K Q^T          [k, q]   (scores, transposed)
      P^T = exp(S^T) * m^T          (mask host-transposed, bf16)
      rowsum = ones^T @ P^T  [1, q] (partition-dim reduce on PE)
      ctx^T = V^T P^T      [d, q]   (V kept [k, d], natural)
      out = ctx_n^T.T @ Wp^T [q, o] (accumulated over heads in SBUF)
  - Softmax without max-subtraction (scores ~ N(0,1), exp is safe) and
    without -inf masking: P = exp(S) * mask, normalized by rowsum(P).
  - Division via reciprocal_approx_fast + gpsimd partition-broadcast.

The head loop is software-pipelined: iteration h emits [proj(h),
outproj(h-1), attn(h)] so the PE never waits on the softmax/normalize
tail of the previous head. Bias matmuls are compiled out when all bias
vectors are zero (they are, for this problem's setup_inputs).
"""
import sys

for _p in (
    "/root/.axon_site",
    "/root/.axon_site/_ro/trn_rl_repo",
    "/root/.axon_site/_ro/pypackages",
):
    if _p not in sys.path:
        sys.path.insert(0, _p)

import numpy as np
import ml_dtypes
from contextlib import ExitStack

import concourse.bacc as bacc
import concourse.tile as tile
import concourse.mybir as mybir
from concourse.bass_utils import run_bass_kernel_spmd

B, L, E, D, H = 8, 1024, 512, 512, 8
NCORES = 8
F32 = mybir.dt.float32
F32R = mybir.dt.float32r
BF16 = mybir.dt.bfloat16
AF = mybir.ActivationFunctionType
SCALE = float(1.0 / np.sqrt(E))

PT_BUFS = 10  # P^T sbuf tiles in flight (8 needed live per (h, qb))

TRACE = False  # test harness sets kernel.TRACE = True to profile
LAST_EXEC_NS = None

_cache = {}


def _build(use_bias):
    nc = bacc.Bacc("TRN2", target_bir_lowering=False, debug=False)

    xT_d = nc.dram_tensor("xT", [4, 128, L], F32R, kind="ExternalInput").ap()
    sT_d = nc.dram_tensor("sT", [4, 128, L], F32R, kind="ExternalInput").ap()
    mk_d = nc.dram_tensor("maskT", [8, 128, L], BF16, kind="ExternalInput").ap()
    wq_d = nc.dram_tensor("wqT", [H, 4, 128, D], F32R, kind="ExternalInput").ap()
    wk_d = nc.dram_tensor("wkT", [H, 4, 128, D], F32R, kind="ExternalInput").ap()
    wv_d = nc.dram_tensor("wvT", [H, 4, 128, D], F32R, kind="ExternalInput").ap()
    wp_d = nc.dram_tensor("wpT", [H, 4, 128, D], F32R, kind="ExternalInput").ap()
    if use_bias:
        bq_d = nc.dram_tensor("bqT", [H, 128, 4], F32, kind="ExternalInput").ap()
        bk_d = nc.dram_tensor("bkT", [H, 128, 4], F32, kind="ExternalInput").ap()
        bv_d = nc.dram_tensor("bv", [H, D], F32R, kind="ExternalInput").ap()
        bp_d = nc.dram_tensor("bp", [1, D], F32R, kind="ExternalInput").ap()
    on_d = nc.dram_tensor("ones", [128, 128], F32R, kind="ExternalInput").ap()
    out_d = nc.dram_tensor("out", [L, D], F32, kind="ExternalOutput").ap()

    with tile.TileContext(nc) as tc, ExitStack() as ctx:
        const = ctx.enter_context(tc.tile_pool(name="const", bufs=1))
        wpool = ctx.enter_context(tc.tile_pool(name="w", bufs=1))
        qkv = ctx.enter_context(tc.tile_pool(name="qkv", bufs=1))
        ptp = ctx.enter_context(tc.tile_pool(name="ptp", bufs=PT_BUFS))
        ctxp = ctx.enter_context(tc.tile_pool(name="ctxp", bufs=1))
        small = ctx.enter_context(tc.tile_pool(name="small", bufs=2))
        psum = ctx.enter_context(tc.tile_pool(name="ps", bufs=7, space="PSUM"))
        psrow = ctx.enter_context(tc.tile_pool(name="psrow", bufs=1, space="PSUM"))

        # Resident tiles. DMA emission order matters for startup latency:
        # head-0 weights + xT/sT go first, bulky mask/bias loads after the
        # first projections are emitted.
        mask_sb = const.tile([128, 8, L], BF16, tag="mask")
        xT = const.tile([128, 4, L], F32R, tag="xT")
        sT = const.tile([128, 4, L], F32R, tag="sT")
        ones = const.tile([128, 128], F32R, tag="ones")
        out_acc = const.tile([128, 8, D], F32, tag="oacc")
        if use_bias:
            bq_sb = const.tile([128, H, 4], F32, tag="bq")
            bk_sb = const.tile([128, H, 4], F32, tag="bk")
            bp_sb = const.tile([1, D], F32R, tag="bp")

        def load_consts():
            """Emitted after proj(0): non-critical-path resident loads."""
            nc.sync.dma_start(mask_sb[:], mk_d.transpose([1, 0, 2]))
            nc.sync.dma_start(ones[:], on_d)
            if use_bias:
                nc.sync.dma_start(bq_sb[:], bq_d.transpose([1, 0, 2]))
                nc.sync.dma_start(bk_sb[:], bk_d.transpose([1, 0, 2]))
                nc.sync.dma_start(bp_sb[:], bp_d)

        state = {}

        def proj(h):
            """Q^T, K^T [128,4dj,L] and V [128,8kj,D] projections for head h."""
            wq = wpool.tile([128, 4, D], F32R, tag="wq")
            wk = wpool.tile([128, 4, D], F32R, tag="wk")
            wv = wpool.tile([128, 4, D], F32R, tag="wv")
            nc.sync.dma_start(wq[:], wq_d[h].transpose([1, 0, 2]))
            if h == 0:
                nc.sync.dma_start(xT[:], xT_d.transpose([1, 0, 2]))
            nc.sync.dma_start(wk[:], wk_d[h].transpose([1, 0, 2]))
            if h == 0:
                nc.sync.dma_start(sT[:], sT_d.transpose([1, 0, 2]))
            nc.sync.dma_start(wv[:], wv_d[h].transpose([1, 0, 2]))
            if use_bias:
                bv_t = small.tile([1, D], F32R, tag="bv")
                nc.sync.dma_start(bv_t[:], bv_d[h : h + 1, :])

            qt = qkv.tile([128, 4, L], F32R, tag="qt")
            kt = qkv.tile([128, 4, L], F32R, tag="kt")
            vt = qkv.tile([128, 8, D], BF16, tag="vt")
            # Q^T / K^T: out[d_tile, q] = sum_e W^T[e, d].T @ xT[e, q]
            for wmat, src, dst, which in (
                (wq, xT, qt, "q"),
                (wk, sT, kt, "k"),
            ):
                for qb in range(2):
                    for dj in range(4):
                        ps = psum.tile([128, 512], F32, tag="mm")
                        for ej in range(4):
                            nc.tensor.matmul(
                                ps[:],
                                wmat[:, ej, dj * 128 : (dj + 1) * 128],
                                src[:, ej, qb * 512 : (qb + 1) * 512],
                                start=(ej == 0),
                                stop=(ej == 3),
                            )
                        dsl = dst[:, dj, qb * 512 : (qb + 1) * 512]
                        if use_bias:
                            bias = bq_sb if which == "q" else bk_sb
                            nc.scalar.activation(
                                dsl, ps[:], AF.Identity,
                                bias=bias[:, h, dj : dj + 1],
                            )
                        else:
                            nc.scalar.copy(dsl, ps[:])
            # V (bf16): out[k_tile, d] = sum_e sT[e, k].T @ Wv^T[e, d] (+ bv)
            for kj in range(8):
                ps = psum.tile([128, 512], F32, tag="mm")
                for ej in range(4):
                    nc.tensor.matmul(
                        ps[:],
                        sT[:, ej, kj * 128 : (kj + 1) * 128],
                        wv[:, ej, :],
                        start=(ej == 0),
                        stop=(ej == 3 and not use_bias),
                    )
                if use_bias:
                    nc.tensor.matmul(
                        ps[:], ones[0:1, :], bv_t[:], start=False, stop=True
                    )
                nc.scalar.copy(vt[:, kj, :], ps[:])
            state[h] = {"qt": qt, "kt": kt, "vt": vt}

        def attn(h):
            """S^T -> exp*mask -> rowsum -> ctx^T -> normalize, per q-block."""
            st = state[h]
            qt, kt, vt = st["qt"], st["kt"], st["vt"]
            ctxn = ctxp.tile([128, 4, L], F32R, tag="ctxn")
            for qb in range(2):
                qsl = slice(qb * 512, (qb + 1) * 512)
                pts = []
                acc = small.tile([128, 512], F32R, tag="acc")
                for kj in range(8):
                    ps = psum.tile([128, 512], F32, tag="mm")
                    for dc in range(4):
                        nc.tensor.matmul(
                            ps[:],
                            kt[:, dc, kj * 128 : (kj + 1) * 128],
                            qt[:, dc, qsl],
                            start=(dc == 0),
                            stop=(dc == 3),
                        )
                    pt = ptp.tile([128, 512], BF16, tag="pt")
                    nc.scalar.activation(pt[:], ps[:], AF.Exp, scale=SCALE)
                    nc.vector.tensor_mul(pt[:], pt[:], mask_sb[:, kj, qsl])
                    if kj == 0:
                        nc.vector.tensor_copy(acc[:], pt[:])
                    else:
                        nc.vector.tensor_add(acc[:], acc[:], pt[:])
                    pts.append(pt)
                rs = psrow.tile([1, 512], F32, tag="row")
                nc.tensor.matmul(
                    rs[:], ones[:, 0:1], acc[:], start=True, stop=True
                )
                rec = small.tile([1, 512], F32, tag="rec")
                nc.vector.reciprocal_approx_fast(rec[:], rs[:])
                rb = small.tile([128, 512], F32, tag="rb")
                nc.gpsimd.partition_broadcast(rb[:], rec[:])
                for dj in range(4):
                    cps = psum.tile([128, 512], F32, tag="mm")
                    for kj in range(8):
                        nc.tensor.matmul(
                            cps[:],
                            vt[:, kj, dj * 128 : (dj + 1) * 128],
                            pts[kj][:],
                            start=(kj == 0),
                            stop=(kj == 7),
                        )
                    nc.vector.tensor_mul(ctxn[:, dj, qsl], cps[:], rb[:])
            state[h]["ctxn"] = ctxn

        def outproj(h):
            """out_acc[q, o] += sum_dj ctx_n^T[i, q].T @ Wp^T[i, o]."""
            wp = wpool.tile([128, 4, D], F32R, tag="wp")
            for dj in range(4):
                nc.sync.dma_start(wp[:, dj, :], wp_d[h, dj])
            ctxn = state[h]["ctxn"]
            for qm in range(8):
                ps = psum.tile([128, 512], F32, tag="mm")
                for dj in range(4):
                    nc.tensor.matmul(
                        ps[:],
                        ctxn[:, dj, qm * 128 : (qm + 1) * 128],
                        wp[:, dj, :],
                        start=(dj == 0),
                        stop=(dj == 3 and not (h == 0 and use_bias)),
                    )
                if h == 0 and use_bias:
                    nc.tensor.matmul(
                        ps[:], ones[0:1, :], bp_sb[:], start=False, stop=True
                    )
                if h == 0:
                    nc.scalar.copy(out_acc[:, qm, :], ps[:])
                else:
                    nc.vector.tensor_add(
                        out_acc[:, qm, :], out_acc[:, qm, :], ps[:]
                    )
                if h == H - 1:
                    nc.sync.dma_start(
                        out_d[qm * 128 : (qm + 1) * 128, :], out_acc[:, qm, :]
                    )
            del state[h]["qt"], state[h]["kt"], state[h]["vt"], state[h]["ctxn"]

        for h in range(H):
            proj(h)
            if h == 0:
                load_consts()
            if h > 0:
                outproj(h - 1)
            attn(h)
        outproj(H - 1)

    nc.compile()
    return nc


def _get_program(use_bias):
    key = ("nc", use_bias)
    if key not in _cache:
        _cache[key] = _build(use_bias)
    return _cache[key]


def kernel(x, states, mask, Wq, bq, Wk, bk, Wv, bv, Wp, bp):
    global LAST_EXEC_NS

    x = np.asarray(x, dtype=np.float32)
    states = np.asarray(states, dtype=np.float32)
    mask = np.asarray(mask)
    f32 = np.float32
    bq_np, bk_np = np.asarray(bq, f32), np.asarray(bk, f32)
    bv_np, bp_np = np.asarray(bv, f32), np.asarray(bp, f32)
    use_bias = bool(
        bq_np.any() or bk_np.any() or bv_np.any() or bp_np.any()
    )
    nc = _get_program(use_bias)

    wq_np = np.ascontiguousarray(
        np.asarray(Wq, f32).transpose(0, 2, 1)
    ).reshape(H, 4, 128, D)
    wk_np = np.ascontiguousarray(
        np.asarray(Wk, f32).transpose(0, 2, 1)
    ).reshape(H, 4, 128, D)
    wv_np = np.ascontiguousarray(
        np.asarray(Wv, f32).transpose(0, 2, 1)
    ).reshape(H, 4, 128, D)
    wp_np = np.ascontiguousarray(np.asarray(Wp, f32).T).reshape(H, 4, 128, D)

    shared = {
        "wqT": wq_np,
        "wkT": wk_np,
        "wvT": wv_np,
        "wpT": wp_np,
        "ones": np.ones((128, 128), f32),
    }
    if use_bias:
        shared["bqT"] = np.ascontiguousarray(
            bq_np.reshape(H, 4, 128).transpose(0, 2, 1)
        )
        shared["bkT"] = np.ascontiguousarray(
            bk_np.reshape(H, 4, 128).transpose(0, 2, 1)
        )
        shared["bv"] = bv_np
        shared["bp"] = bp_np.reshape(1, D)

    in_maps = []
    for b in range(B):
        xT = np.ascontiguousarray(x[b].T).reshape(4, 128, L)
        sT = np.ascontiguousarray(states[b].T).reshape(4, 128, L)
        mT = np.ascontiguousarray(mask[b].T).astype(ml_dtypes.bfloat16).reshape(
            8, 128, L
        )
        in_maps.append({"xT": xT, "sT": sT, "maskT": mT, **shared})

    res = run_bass_kernel_spmd(
        nc, in_maps, core_ids=list(range(NCORES)), trace=TRACE
    )
    LAST_EXEC_NS = res.exec_time_ns
    return np.stack([res.results[b]["out"] for b in range(B)], axis=0)


# revision 22
# speedup vs baseline: 1.2275x; 1.0130x over previous
"""Trainium2 Bass kernel for nn_MultiHeadAttention_64037962383811.

Reference (per batch b):
  q = x @ Wq[h].T + bq[h];  k = states @ Wk[h].T + bk[h];  v = states @ Wv[h].T + bv[h]
  scores = q k^T / sqrt(512);  masked softmax over Lk;  ctx = attn @ v
  out = concat_h(ctx) @ Wp.T + bp

Sharding: data-parallel over batch B=8 -> one batch element per NeuronCore
(8 cores). No collectives; each core computes its full [1024, 512] output
slice and the host stacks them.

Per-core dataflow (all matmuls in float32r = TF32-class, full PE rate):
  - Everything is kept in "transposed" layouts so that the PE's
    partition-dim contraction lines up with zero on-chip transposes:
      x^T, states^T        [e, l]   (host-transposed)
      Q^T, K^T = W^T @ x^T [d, l]   (from projection matmuls directly)
      S^T = K Q^T          [k, q]   (scores, transposed)
      P^T = exp(S^T) * m^T          (mask host-transposed, bf16)
      rowsum = ones^T @ P^T  [1, q] (partition-dim reduce on PE)
      ctx^T = V^T P^T      [d, q]   (V kept [k, d], natural)
      out = ctx_n^T.T @ Wp^T [q, o] (accumulated over heads in SBUF)
  - Softmax without max-subtraction (scores ~ N(0,1), exp is safe) and
    without -inf masking: P = exp(S) * mask, normalized by rowsum(P).
  - Division via reciprocal_approx_fast + gpsimd partition-broadcast.

The head loop is software-pipelined: iteration h emits [proj(h),
outproj(h-1), attn(h)] so the PE never waits on the softmax/normalize
tail of the previous head. Bias matmuls are compiled out when all bias
vectors are zero (they are, for this problem's setup_inputs).
"""
import sys

for _p in (
    "/root/.axon_site",
    "/root/.axon_site/_ro/trn_rl_repo",
    "/root/.axon_site/_ro/pypackages",
):
    if _p not in sys.path:
        sys.path.insert(0, _p)

import numpy as np
import ml_dtypes
from contextlib import ExitStack

import concourse.bacc as bacc
import concourse.tile as tile
import concourse.mybir as mybir
from concourse.bass_utils import run_bass_kernel_spmd

B, L, E, D, H = 8, 1024, 512, 512, 8
NCORES = 8
F32 = mybir.dt.float32
F32R = mybir.dt.float32r
BF16 = mybir.dt.bfloat16
AF = mybir.ActivationFunctionType
SCALE = float(1.0 / np.sqrt(E))

PT_BUFS = 10  # P^T sbuf tiles in flight (8 needed live per (h, qb))

TRACE = False  # test harness sets kernel.TRACE = True to profile
LAST_EXEC_NS = None

_cache = {}


def _build(use_bias):
    nc = bacc.Bacc("TRN2", target_bir_lowering=False, debug=False)

    xT_d = nc.dram_tensor("xT", [4, 128, L], F32R, kind="ExternalInput").ap()
    sT_d = nc.dram_tensor("sT", [4, 128, L], F32R, kind="ExternalInput").ap()
    mk_d = nc.dram_tensor("maskT", [8, 128, L], BF16, kind="ExternalInput").ap()
    wq_d = nc.dram_tensor("wqT", [H, 4, 128, D], F32R, kind="ExternalInput").ap()
    wk_d = nc.dram_tensor("wkT", [H, 4, 128, D], F32R, kind="ExternalInput").ap()
    wv_d = nc.dram_tensor("wvT", [H, 4, 128, D], F32R, kind="ExternalInput").ap()
    wp_d = nc.dram_tensor("wpT", [H, 4, 128, D], F32R, kind="ExternalInput").ap()
    if use_bias:
        bq_d = nc.dram_tensor("bqT", [H, 128, 4], F32, kind="ExternalInput").ap()
        bk_d = nc.dram_tensor("bkT", [H, 128, 4], F32, kind="ExternalInput").ap()
        bv_d = nc.dram_tensor("bv", [H, D], F32R, kind="ExternalInput").ap()
        bp_d = nc.dram_tensor("bp", [1, D], F32R, kind="ExternalInput").ap()
    on_d = nc.dram_tensor("ones", [128, 128], F32R, kind="ExternalInput").ap()
    out_d = nc.dram_tensor("out", [L, D], F32, kind="ExternalOutput").ap()

    with tile.TileContext(nc) as tc, ExitStack() as ctx:
        const = ctx.enter_context(tc.tile_pool(name="const", bufs=1))
        wpool = ctx.enter_context(tc.tile_pool(name="w", bufs=1))
        qkv = ctx.enter_context(tc.tile_pool(name="qkv", bufs=1))
        ptp = ctx.enter_context(tc.tile_pool(name="ptp", bufs=PT_BUFS))
        ctxp = ctx.enter_context(tc.tile_pool(name="ctxp", bufs=1))
        small = ctx.enter_context(tc.tile_pool(name="small", bufs=2))
        psum = ctx.enter_context(tc.tile_pool(name="ps", bufs=7, space="PSUM"))
        psrow = ctx.enter_context(tc.tile_pool(name="psrow", bufs=1, space="PSUM"))

        # Resident tiles. DMA emission order matters for startup latency:
        # head-0 weights + xT/sT go first, bulky mask/bias loads after the
        # first projections are emitted.
        mask_sb = const.tile([128, 8, L], BF16, tag="mask")
        xT = const.tile([128, 4, L], F32R, tag="xT")
        sT = const.tile([128, 4, L], F32R, tag="sT")
        ones = const.tile([128, 128], F32R, tag="ones")
        out_acc = const.tile([128, 8, D], F32, tag="oacc")
        if use_bias:
            bq_sb = const.tile([128, H, 4], F32, tag="bq")
            bk_sb = const.tile([128, H, 4], F32, tag="bk")
            bp_sb = const.tile([1, D], F32R, tag="bp")

        def load_consts():
            """Emitted after proj(0): non-critical-path resident loads."""
            nc.sync.dma_start(mask_sb[:], mk_d.transpose([1, 0, 2]))
            nc.sync.dma_start(ones[:], on_d)
            if use_bias:
                nc.sync.dma_start(bq_sb[:], bq_d.transpose([1, 0, 2]))
                nc.sync.dma_start(bk_sb[:], bk_d.transpose([1, 0, 2]))
                nc.sync.dma_start(bp_sb[:], bp_d)

        state = {}

        def proj(h):
            """Q^T, K^T [128,4dj,L] and V [128,8kj,D] projections for head h."""
            wq = wpool.tile([128, 4, D], F32R, tag="wq")
            wk = wpool.tile([128, 4, D], F32R, tag="wk")
            wv = wpool.tile([128, 4, D], F32R, tag="wv")
            if h == 0:
                # Fine-grained first loads: the first projection matmul only
                # needs (wq, xT) slab ej=0, so don't gate it on 3 MB of DMA.
                for ej in range(4):
                    nc.sync.dma_start(wq[:, ej, :], wq_d[h, ej])
                    nc.sync.dma_start(xT[:, ej, :], xT_d[ej])
                for ej in range(4):
                    nc.sync.dma_start(wk[:, ej, :], wk_d[h, ej])
                    nc.sync.dma_start(sT[:, ej, :], sT_d[ej])
            else:
                nc.sync.dma_start(wq[:], wq_d[h].transpose([1, 0, 2]))
                nc.sync.dma_start(wk[:], wk_d[h].transpose([1, 0, 2]))
            nc.sync.dma_start(wv[:], wv_d[h].transpose([1, 0, 2]))
            if use_bias:
                bv_t = small.tile([1, D], F32R, tag="bv")
                nc.sync.dma_start(bv_t[:], bv_d[h : h + 1, :])

            qt = qkv.tile([128, 4, L], F32R, tag="qt")
            kt = qkv.tile([128, 4, L], F32R, tag="kt")
            vt = qkv.tile([128, 8, D], BF16, tag="vt")
            # Q^T / K^T: out[d_tile, q] = sum_e W^T[e, d].T @ xT[e, q]
            for wmat, src, dst, which in (
                (wq, xT, qt, "q"),
                (wk, sT, kt, "k"),
            ):
                for qb in range(2):
                    for dj in range(4):
                        ps = psum.tile([128, 512], F32, tag="mm")
                        for ej in range(4):
                            nc.tensor.matmul(
                                ps[:],
                                wmat[:, ej, dj * 128 : (dj + 1) * 128],
                                src[:, ej, qb * 512 : (qb + 1) * 512],
                                start=(ej == 0),
                                stop=(ej == 3),
                            )
                        dsl = dst[:, dj, qb * 512 : (qb + 1) * 512]
                        if use_bias:
                            bias = bq_sb if which == "q" else bk_sb
                            nc.scalar.activation(
                                dsl, ps[:], AF.Identity,
                                bias=bias[:, h, dj : dj + 1],
                            )
                        else:
                            nc.scalar.copy(dsl, ps[:])
            # V (bf16): out[k_tile, d] = sum_e sT[e, k].T @ Wv^T[e, d] (+ bv)
            for kj in range(8):
                ps = psum.tile([128, 512], F32, tag="mm")
                for ej in range(4):
                    nc.tensor.matmul(
                        ps[:],
                        sT[:, ej, kj * 128 : (kj + 1) * 128],
                        wv[:, ej, :],
                        start=(ej == 0),
                        stop=(ej == 3 and not use_bias),
                    )
                if use_bias:
                    nc.tensor.matmul(
                        ps[:], ones[0:1, :], bv_t[:], start=False, stop=True
                    )
                nc.scalar.copy(vt[:, kj, :], ps[:])
            state[h] = {"qt": qt, "kt": kt, "vt": vt}

        def attn(h):
            """S^T -> exp*mask -> rowsum -> ctx^T -> normalize, per q-block."""
            st = state[h]
            qt, kt, vt = st["qt"], st["kt"], st["vt"]
            ctxn = ctxp.tile([128, 4, L], F32R, tag="ctxn")
            for qb in range(2):
                qsl = slice(qb * 512, (qb + 1) * 512)
                pts = []
                acc = small.tile([128, 512], F32R, tag="acc")
                for kj in range(8):
                    ps = psum.tile([128, 512], F32, tag="mm")
                    for dc in range(4):
                        nc.tensor.matmul(
                            ps[:],
                            kt[:, dc, kj * 128 : (kj + 1) * 128],
                            qt[:, dc, qsl],
                            start=(dc == 0),
                            stop=(dc == 3),
                        )
                    pt = ptp.tile([128, 512], BF16, tag="pt")
                    nc.scalar.activation(pt[:], ps[:], AF.Exp, scale=SCALE)
                    nc.vector.tensor_mul(pt[:], pt[:], mask_sb[:, kj, qsl])
                    if kj == 0:
                        nc.vector.tensor_copy(acc[:], pt[:])
                    else:
                        nc.vector.tensor_add(acc[:], acc[:], pt[:])
                    pts.append(pt)
                rs = psrow.tile([1, 512], F32, tag="row")
                nc.tensor.matmul(
                    rs[:], ones[:, 0:1], acc[:], start=True, stop=True
                )
                rec = small.tile([1, 512], F32, tag="rec")
                nc.vector.reciprocal_approx_fast(rec[:], rs[:])
                rb = small.tile([128, 512], F32, tag="rb")
                nc.gpsimd.partition_broadcast(rb[:], rec[:])
                for dj in range(4):
                    cps = psum.tile([128, 512], F32, tag="mm")
                    for kj in range(8):
                        nc.tensor.matmul(
                            cps[:],
                            vt[:, kj, dj * 128 : (dj + 1) * 128],
                            pts[kj][:],
                            start=(kj == 0),
                            stop=(kj == 7),
                        )
                    nc.vector.tensor_mul(ctxn[:, dj, qsl], cps[:], rb[:])
            state[h]["ctxn"] = ctxn

        def outproj(h):
            """out_acc[q, o] += sum_dj ctx_n^T[i, q].T @ Wp^T[i, o]."""
            wp = wpool.tile([128, 4, D], F32R, tag="wp")
            for dj in range(4):
                nc.sync.dma_start(wp[:, dj, :], wp_d[h, dj])
            ctxn = state[h]["ctxn"]
            for qm in range(8):
                ps = psum.tile([128, 512], F32, tag="mm")
                for dj in range(4):
                    nc.tensor.matmul(
                        ps[:],
                        ctxn[:, dj, qm * 128 : (qm + 1) * 128],
                        wp[:, dj, :],
                        start=(dj == 0),
                        stop=(dj == 3 and not (h == 0 and use_bias)),
                    )
                if h == 0 and use_bias:
                    nc.tensor.matmul(
                        ps[:], ones[0:1, :], bp_sb[:], start=False, stop=True
                    )
                if h == 0:
                    nc.scalar.copy(out_acc[:, qm, :], ps[:])
                else:
                    nc.vector.tensor_add(
                        out_acc[:, qm, :], out_acc[:, qm, :], ps[:]
                    )
                if h == H - 1:
                    nc.sync.dma_start(
                        out_d[qm * 128 : (qm + 1) * 128, :], out_acc[:, qm, :]
                    )
            del state[h]["qt"], state[h]["kt"], state[h]["vt"], state[h]["ctxn"]

        for h in range(H):
            proj(h)
            if h == 0:
                load_consts()
            if h > 0:
                outproj(h - 1)
            attn(h)
        outproj(H - 1)

    nc.compile()
    return nc


def _get_program(use_bias):
    key = ("nc", use_bias)
    if key not in _cache:
        _cache[key] = _build(use_bias)
    return _cache[key]


def kernel(x, states, mask, Wq, bq, Wk, bk, Wv, bv, Wp, bp):
    global LAST_EXEC_NS

    x = np.asarray(x, dtype=np.float32)
    states = np.asarray(states, dtype=np.float32)
    mask = np.asarray(mask)
    f32 = np.float32
    bq_np, bk_np = np.asarray(bq, f32), np.asarray(bk, f32)
    bv_np, bp_np = np.asarray(bv, f32), np.asarray(bp, f32)
    use_bias = bool(
        bq_np.any() or bk_np.any() or bv_np.any() or bp_np.any()
    )
    nc = _get_program(use_bias)

    wq_np = np.ascontiguousarray(
        np.asarray(Wq, f32).transpose(0, 2, 1)
    ).reshape(H, 4, 128, D)
    wk_np = np.ascontiguousarray(
        np.asarray(Wk, f32).transpose(0, 2, 1)
    ).reshape(H, 4, 128, D)
    wv_np = np.ascontiguousarray(
        np.asarray(Wv, f32).transpose(0, 2, 1)
    ).reshape(H, 4, 128, D)
    wp_np = np.ascontiguousarray(np.asarray(Wp, f32).T).reshape(H, 4, 128, D)

    shared = {
        "wqT": wq_np,
        "wkT": wk_np,
        "wvT": wv_np,
        "wpT": wp_np,
        "ones": np.ones((128, 128), f32),
    }
    if use_bias:
        shared["bqT"] = np.ascontiguousarray(
            bq_np.reshape(H, 4, 128).transpose(0, 2, 1)
        )
        shared["bkT"] = np.ascontiguousarray(
            bk_np.reshape(H, 4, 128).transpose(0, 2, 1)
        )
        shared["bv"] = bv_np
        shared["bp"] = bp_np.reshape(1, D)

    in_maps = []
    for b in range(B):
        xT = np.ascontiguousarray(x[b].T).reshape(4, 128, L)
        sT = np.ascontiguousarray(states[b].T).reshape(4, 128, L)
        mT = np.ascontiguousarray(mask[b].T).astype(ml_dtypes.bfloat16).reshape(
            8, 128, L
        )
        in_maps.append({"xT": xT, "sT": sT, "maskT": mT, **shared})

    res = run_bass_kernel_spmd(
        nc, in_maps, core_ids=list(range(NCORES)), trace=TRACE
    )
    LAST_EXEC_NS = res.exec_time_ns
    return np.stack([res.results[b]["out"] for b in range(B)], axis=0)
